# revision 1
# baseline (speedup 1.0000x reference)
"""GCN 2-layer kernel for trn2 (8 NeuronCores, SPMD).

Sharding: nodes dst-sharded across 8 cores (12500 each, padded to 12544).
Each core owns the edges whose dst lands in its shard (plus self-loops),
packed into 128-edge chunks, 37 chunks per 128-dst block (static layout).
Layer tables (g = dinv * (x@W+b)) are built per-shard on the TensorEngine
and AllGathered. Messages are fetched with dma_gather (4-row packs keep
indices in int16), lane-selected on DVE, and scatter-added per dst-block
with one-hot matmuls on the TensorEngine accumulating in PSUM.
log_softmax runs on-device; host only shards/permutes and concatenates.
"""
import sys
import numpy as np

if "/opt/trn_rl_repo" not in sys.path:
    sys.path.insert(0, "/opt/trn_rl_repo")

N = 100000
NCORES = 8
SHARD = 12500
SHARD_PAD = 12544          # 128 * 98
NBLK = 98                  # dst blocks of 128 per core
BPC = 37                   # chunks per block (128 edges each)
NCHUNK = NBLK * BPC        # 3626 PE chunks
NI = 1024                  # edges per dma_gather instruction (8 chunks)
NGI = (NCHUNK + 7) // 8    # 454 gather instructions
NCHUNK_G = NGI * 8         # 3632 chunk slots incl. trailing pads
NFEAT = 512
NHID = 16
NCLASS = 40
CPAD = 48                  # padded class dim
PAD_PACK = 3125            # pack of rows 12500..12503 of core 0 (always zero)
WCOL = NI // 16            # gidx columns per gather instruction (64)

_PROGRAM = None


def _build_program():
    import concourse.bacc as bacc
    import concourse.mybir as mybir
    import concourse.tile as tile

    f32 = mybir.dt.float32
    bf16 = mybir.dt.bfloat16
    i16 = mybir.dt.int16
    Alu = mybir.AluOpType
    Act = mybir.ActivationFunctionType
    Axis = mybir.AxisListType

    nc = bacc.Bacc("TRN2", target_bir_lowering=False, debug=False,
                   num_devices=NCORES, num_swdge_queues=4)

    xT = nc.dram_tensor("xT", [NFEAT, SHARD_PAD], f32, kind="ExternalInput")
    rp0 = nc.dram_tensor("rp0", [128, NBLK], f32, kind="ExternalInput")
    rp1 = nc.dram_tensor("rp1", [128, NBLK], f32, kind="ExternalInput")
    gidx = nc.dram_tensor("gidx", [128, NGI * WCOL], i16, kind="ExternalInput")
    lane = nc.dram_tensor("lane", [128, NCHUNK_G], f32, kind="ExternalInput")
    dstl = nc.dram_tensor("dstl", [128, NCHUNK_G], f32, kind="ExternalInput")
    w1 = nc.dram_tensor("w1", [NFEAT, NHID], f32, kind="ExternalInput")
    b1 = nc.dram_tensor("b1", [1, NHID], f32, kind="ExternalInput")
    w2 = nc.dram_tensor("w2", [NHID, CPAD], f32, kind="ExternalInput")
    b2 = nc.dram_tensor("b2", [1, CPAD], f32, kind="ExternalInput")
    iota8 = nc.dram_tensor("iota8", [128, 8 * 128], f32, kind="ExternalInput")
    ident = nc.dram_tensor("ident", [128, 128], f32, kind="ExternalInput")
    padmask = nc.dram_tensor("padmask", [128, 1], f32, kind="ExternalInput")
    out = nc.dram_tensor("out", [SHARD_PAD, NCLASS], f32,
                         kind="ExternalOutput")

    with tile.TileContext(nc) as tc:
        with (
            tc.tile_pool(name="const", bufs=1) as constp,
            tc.tile_pool(name="stream", bufs=4) as streamp,
            tc.tile_pool(name="gat", bufs=6) as gatp,
            tc.tile_pool(name="work", bufs=4) as workp,
            tc.tile_pool(name="fin", bufs=3) as finp,
            tc.tile_pool(name="psum", bufs=2, space="PSUM") as psump,
            tc.tile_pool(name="dram", bufs=1, space="DRAM") as dramp,
        ):
            # ---- constants ----
            iota_t = constp.tile([128, 8 * 128], f32)
            nc.sync.dma_start(iota_t[:], iota8[:])
            ident_t = constp.tile([128, 128], f32)
            nc.sync.dma_start(ident_t[:], ident[:])
            w1_t = constp.tile([128, NFEAT // 128, NHID], f32)
            nc.sync.dma_start(w1_t[:],
                              w1[:].rearrange("(a k) h -> k a h", k=128))
            b1_t = constp.tile([1, NHID], f32)
            nc.sync.dma_start(b1_t[:], b1[:])
            w2_t = constp.tile([NHID, CPAD], f32)
            nc.sync.dma_start(w2_t[:], w2[:])
            b2_t = constp.tile([1, CPAD], f32)
            nc.sync.dma_start(b2_t[:], b2[:])
            ones_t = constp.tile([1, 128], f32)
            nc.vector.memset(ones_t[:], 1.0)
            lane_t = constp.tile([128, NCHUNK_G], f32)
            nc.sync.dma_start(lane_t[:], lane[:])
            dstl_t = constp.tile([128, NCHUNK_G], f32)
            nc.sync.dma_start(dstl_t[:], dstl[:])

            # ---- dinv[p, c] for node 128c+p: rsqrt(deg + 1) ----
            dinv_t = constp.tile([128, NBLK], f32)
            rp0_t = workp.tile([128, NBLK], f32, tag="rp")
            rp1_t = workp.tile([128, NBLK], f32, tag="rp")
            nc.sync.dma_start(rp0_t[:], rp0[:])
            nc.sync.dma_start(rp1_t[:], rp1[:])
            deg_t = workp.tile([128, NBLK], f32, tag="deg")
            nc.vector.tensor_tensor(out=deg_t[:], in0=rp1_t[:], in1=rp0_t[:],
                                    op=Alu.subtract)
            sqd_t = workp.tile([128, NBLK], f32, tag="deg")
            nc.scalar.activation(out=sqd_t[:], in_=deg_t[:], func=Act.Sqrt,
                                 bias=1.0)
            nc.vector.reciprocal(out=dinv_t[:], in_=sqd_t[:])
            pm_t = constp.tile([128, 1], f32)
            nc.sync.dma_start(pm_t[:], padmask[:])
            dinv97_t = constp.tile([128, 1], f32)
            nc.vector.tensor_tensor(out=dinv97_t[:],
                                    in0=dinv_t[:, NBLK - 1:NBLK],
                                    in1=pm_t[:], op=Alu.mult)

            def dcol_of(b):
                return dinv97_t[:] if b == NBLK - 1 else dinv_t[:, b:b + 1]

            # ---- DRAM tables ----
            t1shard = dramp.tile([SHARD_PAD, NHID], f32)
            t1full = dramp.tile([SHARD_PAD * NCORES, NHID], f32)
            t2shard = dramp.tile([SHARD_PAD, 64], bf16)
            t2full = dramp.tile([SHARD_PAD * NCORES, 64], bf16)

            # ---- phase 1: table1 rows = dinv * (x @ W1 + b1) ----
            for c in range(NBLK):
                ph = psump.tile([128, NHID], f32, tag="ph1")
                for kk in range(NFEAT // 128):
                    xk = streamp.tile([128, 128], f32, tag="xk")
                    nc.sync.dma_start(
                        xk[:],
                        xT[kk * 128:(kk + 1) * 128, c * 128:(c + 1) * 128])
                    nc.tensor.matmul(out=ph[:], lhsT=xk[:],
                                     rhs=w1_t[:, kk, :],
                                     start=(kk == 0), stop=False)
                nc.tensor.matmul(out=ph[:], lhsT=ones_t[:], rhs=b1_t[:],
                                 start=False, stop=True)
                hs = workp.tile([128, NHID], f32, tag="hs1")
                nc.scalar.activation(out=hs[:], in_=ph[:], func=Act.Copy,
                                     scale=dcol_of(c))
                nc.sync.dma_start(t1shard[c * 128:(c + 1) * 128, :], hs[:])

            nc.gpsimd.collective_compute(
                "AllGather", Alu.bypass,
                ins=[t1shard[:].opt()], outs=[t1full[:].opt()],
                replica_groups=[list(range(NCORES))],
            )
            t1packs = t1full[:].rearrange("(q r) h -> q (r h)", r=4)

            # ---- message-passing phase builder ----
            def finish_block1(b, acc):
                a1 = finp.tile([128, NHID], f32, tag="a1")
                nc.scalar.activation(out=a1[:], in_=acc[:], func=Act.Relu,
                                     scale=dcol_of(b))
                pt = psump.tile([NHID, 128], f32, tag="ptr")
                nc.tensor.transpose(out=pt[:], in_=a1[:], identity=ident_t[:])
                a1T = finp.tile([NHID, 128], f32, tag="a1T")
                nc.vector.tensor_copy(out=a1T[:], in_=pt[:])
                ph2 = psump.tile([128, CPAD], f32, tag="ph2")
                nc.tensor.matmul(out=ph2[:], lhsT=a1T[:], rhs=w2_t[:],
                                 start=True, stop=False)
                nc.tensor.matmul(out=ph2[:], lhsT=ones_t[:], rhs=b2_t[:],
                                 start=False, stop=True)
                h2 = finp.tile([128, 64], bf16, tag="h2")
                nc.vector.memset(h2[:, CPAD:], 0.0)
                nc.scalar.activation(out=h2[:, :CPAD], in_=ph2[:],
                                     func=Act.Copy, scale=dcol_of(b))
                nc.sync.dma_start(t2shard[b * 128:(b + 1) * 128, :], h2[:])

            def finish_block2(b, acc):
                o2 = finp.tile([128, CPAD], f32, tag="o2")
                nc.scalar.activation(out=o2[:], in_=acc[:], func=Act.Copy,
                                     scale=dcol_of(b))
                rmax = finp.tile([128, 1], f32, tag="rmax")
                nc.vector.tensor_reduce(out=rmax[:], in_=o2[:, :NCLASS],
                                        axis=Axis.X, op=Alu.max)
                sh = finp.tile([128, NCLASS], f32, tag="sh")
                nc.vector.tensor_scalar(out=sh[:], in0=o2[:, :NCLASS],
                                        scalar1=rmax[:], scalar2=None,
                                        op0=Alu.subtract)
                ex = finp.tile([128, NCLASS], f32, tag="ex")
                nc.scalar.activation(out=ex[:], in_=sh[:], func=Act.Exp)
                rsum = finp.tile([128, 1], f32, tag="rsum")
                nc.vector.tensor_reduce(out=rsum[:], in_=ex[:],
                                        axis=Axis.X, op=Alu.add)
                lsum = finp.tile([128, 1], f32, tag="lsum")
                nc.scalar.activation(out=lsum[:], in_=rsum[:], func=Act.Ln)
                res = finp.tile([128, NCLASS], f32, tag="res")
                nc.vector.tensor_scalar(out=res[:], in0=sh[:],
                                        scalar1=lsum[:], scalar2=None,
                                        op0=Alu.subtract)
                nc.sync.dma_start(out[b * 128:(b + 1) * 128, :], res[:])

            def mp_phase(packs_ap, elem, rowlen, fdim, gat_dt, finish, tagsfx):
                acc_holder = [None]
                for gi in range(NGI):
                    gx = gatp.tile([128, WCOL], i16, tag="gx" + tagsfx)
                    nc.sync.dma_start(gx[:],
                                      gidx[:, gi * WCOL:(gi + 1) * WCOL])
                    gat = gatp.tile([128, 8, elem], gat_dt, tag="gt" + tagsfx)
                    nc.gpsimd.dma_gather(gat[:], packs_ap, gx[:], NI, NI,
                                         elem, queue_num=gi % 4)
                    cs = slice(gi * 8, gi * 8 + 8)
                    msg = gatp.tile([128, 8, fdim], f32, tag="ms" + tagsfx)
                    for ll in range(4):
                        mk = workp.tile([128, 8], f32, tag="mk" + tagsfx)
                        nc.vector.tensor_scalar(out=mk[:], in0=lane_t[:, cs],
                                                scalar1=float(ll),
                                                scalar2=None,
                                                op0=Alu.is_equal)
                        sel = gat[:, :, rowlen * ll:rowlen * ll + fdim]
                        mkb = mk[:].to_broadcast([128, 8, fdim])
                        if ll == 0:
                            nc.vector.tensor_tensor(out=msg[:], in0=sel,
                                                    in1=mkb, op=Alu.mult)
                        else:
                            tmp = workp.tile([128, 8, fdim], f32,
                                             tag="tp" + tagsfx)
                            nc.vector.tensor_tensor(out=tmp[:], in0=sel,
                                                    in1=mkb, op=Alu.mult)
                            nc.any.tensor_tensor(out=msg[:], in0=msg[:],
                                                 in1=tmp[:], op=Alu.add)
                    onehot = gatp.tile([128, 8, 128], f32, tag="oh" + tagsfx)
                    dcol = dstl_t[:, cs].to_broadcast([128, 8, 128])
                    nc.vector.tensor_tensor(
                        out=onehot[:], in0=dcol,
                        in1=iota_t[:].rearrange("p (c e) -> p c e", e=128),
                        op=Alu.is_equal)
                    for j in range(8):
                        c = gi * 8 + j
                        if c >= NCHUNK:
                            continue
                        b, jj = divmod(c, BPC)
                        if jj == 0:
                            acc_holder[0] = psump.tile(
                                [128, fdim], f32, tag="acc", name="acc_t")
                        nc.tensor.matmul(out=acc_holder[0][:],
                                         lhsT=onehot[:, j, :],
                                         rhs=msg[:, j, :],
                                         start=(jj == 0),
                                         stop=(jj == BPC - 1))
                        if jj == BPC - 1:
                            finish(b, acc_holder[0])

            # ---- phase 2: layer 1 edges ----
            mp_phase(t1packs, 64, NHID, NHID, f32, finish_block1, "1")

            nc.gpsimd.collective_compute(
                "AllGather", Alu.bypass,
                ins=[t2shard[:].opt()], outs=[t2full[:].opt()],
                replica_groups=[list(range(NCORES))],
            )
            t2packs = t2full[:].rearrange("(q r) h -> q (r h)", r=4)

            # ---- phase 4: layer 2 edges ----
            mp_phase(t2packs, 256, 64, CPAD, bf16, finish_block2, "2")

    nc.compile()
    return nc


def _host_prep(x, edge_index, W1, b1, W2, b2):
    src = np.asarray(edge_index[0], dtype=np.int64)
    dst = np.asarray(edge_index[1], dtype=np.int64)

    counts = np.bincount(src, minlength=N)
    rowptr = np.zeros(N + 1, dtype=np.int64)
    np.cumsum(counts, out=rowptr[1:])

    iota8 = np.tile(np.tile(np.arange(128, dtype=np.float32), 8), (128, 1))
    pmk = np.ones((128, 1), dtype=np.float32)
    pmk[84:, 0] = 0.0
    ident = np.eye(128, dtype=np.float32)
    w2p = np.zeros((NHID, CPAD), dtype=np.float32)
    w2p[:, :NCLASS] = W2
    b2p = np.zeros((1, CPAD), dtype=np.float32)
    b2p[0, :NCLASS] = b2

    dst_core = dst // SHARD
    in_maps = []
    for k in range(NCORES):
        ids = np.arange(SHARD, dtype=np.int64) + k * SHARD
        xT = np.zeros((NFEAT, SHARD_PAD), dtype=np.float32)
        xT[:, :SHARD] = x[ids].T

        rpl = np.zeros(SHARD_PAD, dtype=np.float32)
        rph = np.zeros(SHARD_PAD, dtype=np.float32)
        rpl[:SHARD] = rowptr[ids]
        rph[:SHARD] = rowptr[ids + 1]
        rp0 = np.ascontiguousarray(rpl.reshape(NBLK, 128).T)
        rp1 = np.ascontiguousarray(rph.reshape(NBLK, 128).T)

        m = dst_core == k
        es = np.concatenate([src[m], ids])
        ed = np.concatenate([dst[m] - k * SHARD, ids - k * SHARD])
        order = np.argsort(ed, kind="stable")
        es, ed = es[order], ed[order]

        blocks = ed >> 7
        bcnt = np.bincount(blocks, minlength=NBLK)
        assert bcnt.max() <= BPC * 128, f"block overflow: {bcnt.max()}"
        bstart = np.zeros(NBLK, dtype=np.int64)
        np.cumsum(bcnt[:-1], out=bstart[1:])
        pos = np.arange(len(es)) - bstart[blocks]
        slot_c = BPC * blocks + (pos >> 7)
        slot_p = pos & 127

        row = (es // SHARD) * SHARD_PAD + es % SHARD
        pack = np.full((128, NCHUNK_G), PAD_PACK, dtype=np.int64)
        lanev = np.zeros((128, NCHUNK_G), dtype=np.float32)
        dstlv = np.zeros((128, NCHUNK_G), dtype=np.float32)
        pack[slot_p, slot_c] = row >> 2
        lanev[slot_p, slot_c] = (row & 3).astype(np.float32)
        dstlv[slot_p, slot_c] = (ed & 127).astype(np.float32)

        gx = np.empty((128, NGI * WCOL), dtype=np.int16)
        for gi in range(NGI):
            blk8 = pack[:, gi * 8:(gi + 1) * 8]        # [128 p, 8 c]
            idx_list = blk8.T.ravel()                   # q = c*128 + p
            wrapped = idx_list.reshape(WCOL, 16).T      # [16, 64]
            gx[:, gi * WCOL:(gi + 1) * WCOL] = np.tile(
                wrapped, (8, 1)).astype(np.int16)

        in_maps.append({
            "xT": xT, "rp0": rp0, "rp1": rp1, "gidx": gx,
            "lane": lanev, "dstl": dstlv,
            "w1": np.ascontiguousarray(W1),
            "b1": b1.reshape(1, NHID).astype(np.float32),
            "w2": w2p, "b2": b2p,
            "iota8": iota8, "ident": ident, "padmask": pmk,
        })
    return in_maps


def kernel(**inputs):
    global _PROGRAM
    x = np.asarray(inputs["x"], dtype=np.float32)
    edge_index = np.asarray(inputs["edge_index"])
    W1 = np.asarray(inputs["W1"], dtype=np.float32)
    b1 = np.asarray(inputs["b1"], dtype=np.float32)
    W2 = np.asarray(inputs["W2"], dtype=np.float32)
    b2 = np.asarray(inputs["b2"], dtype=np.float32)

    in_maps = _host_prep(x, edge_index, W1, b1, W2, b2)

    if _PROGRAM is None:
        _PROGRAM = _build_program()

    from concourse import bass_utils
    res = bass_utils.run_bass_kernel_spmd(
        _PROGRAM, in_maps, core_ids=list(range(NCORES)))
    parts = [np.asarray(res.results[k]["out"])[:SHARD] for k in range(NCORES)]
    return np.concatenate(parts, axis=0).astype(np.float32)

